# revision 4
# baseline (speedup 1.0000x reference)
"""Bass/Tile TRN2 kernel for nn_Attention (additive/Bahdanau-style attention).

reference math per batch b:
  res_q = query[b] @ W_q.T                      (Q, H)
  res_c = context[b] @ W_c.T + b_c              (C, H)
  logit[q,c] = sum_h W_o[h]*tanh(res_c[c,h] + res_q[q,h]) + b_o
  w = mask * exp(logit); weights = w / (sum_c w + eps)
  out = weights @ context[b]

The (Q,C,H) tanh grid is never materialized. tanh is replaced by an
M-term harmonic sine series  tanh(x) ~= k*x + sum_m c_m sin(m*w0*x), and
the angle-addition identity factorizes each term so the whole logit is
ONE PE contraction over (h, m, trig) chunks of 128:
  logit[q,c] = sum_f B_f[h,q] * A_f[h,c]
b_c folds into the rq staging copy (ACT per-partition bias), so the
A(context) side is bias-free.  Only the fundamental sin/cos touch the
ACT Sin table (cos through one DVE add_range_wrap with the pi/2 phase
folded into the wrap shift); harmonics 2..3 come from double/triple
angle products (squares on ACT, the rest on DVE), with the 2x factor of
s2'=sin2/2 folded into the host-side W_o*c_m coefficients.  All B-side
(query, 64-col) products and folds run on the Pool engine so the DVE
only carries the wide A-side chain.  A rank-1 ln(mask) chunk folds the
mask into the logit so exp's accum_out directly yields the masked
softmax denominator.  The PE transpose of the exp uses diag(1/rowsum)
instead of the identity, so the final weights @ context matmul emits
already-normalized output.

All matmul operands are bf16; PSUM accumulation stays f32.  Inputs ride
the two hardware-DGE DMA queues (sync, scalar), balanced so res_q and
res_c start as early as possible; res_c chases the ctxT chunks k-outer.

Sharding: data-parallel over batch B=8 across the 8 NeuronCores.
"""

import numpy as np

B, Q, C, D, H = 8, 64, 512, 512, 256
EPS = 1e-5
P = 128
KD = D // P   # 4 chunks of the contraction dim d
KC = C // P   # 4 chunks of the context dim c
JH = H // P   # 2 chunks of the hidden dim h
N_CORES = 8

# tanh(x) ~= K_LIN*x + sum_m CS[m]*sin(m*W0*x), fitted on [-4.9, 4.9]
M_HARM = 3
W0 = 0.90
K_LIN = 0.2846621628161513
CS = [0.4703299122559558, 0.1052940908727595, 0.02379076660069648]
PI = float(np.pi)
TRIM = 1.0 - 5e-7    # keeps |w0*x| strictly inside the Sin table domain


def _build_program(b_o_val: float):
    import concourse.bacc as bacc
    import concourse.mybir as mybir
    import concourse.tile as tile
    from concourse.alu_op_type import AluOpType
    from concourse import masks
    from contextlib import ExitStack

    F32 = mybir.dt.float32
    BF16 = mybir.dt.bfloat16
    Act = mybir.ActivationFunctionType

    nc = bacc.Bacc("TRN2", target_bir_lowering=False, debug=False)

    WqT_d = nc.dram_tensor("WqT", [D, H], BF16, kind="ExternalInput")
    WcT_d = nc.dram_tensor("WcT", [D, H], BF16, kind="ExternalInput")
    qT_d = nc.dram_tensor("qT", [D, Q], BF16, kind="ExternalInput")
    ctxT_d = nc.dram_tensor("ctxT", [D, C], BF16, kind="ExternalInput")
    ctx_d = nc.dram_tensor("ctx", [C, D], BF16, kind="ExternalInput")
    mrow_d = nc.dram_tensor("mrow", [1, C], BF16, kind="ExternalInput")
    # cols 0..M-1: W_o * c_m folds (2x on m=2 for the halved sin2 product);
    # col M: W_o * k_lin; col M+1: b_c
    WoCK_d = nc.dram_tensor("WoCK", [P, JH, M_HARM + 2], F32, kind="ExternalInput")
    out_d = nc.dram_tensor("out", [Q, D], F32, kind="ExternalOutput")
    wts_d = nc.dram_tensor("wts", [Q, C], F32, kind="ExternalOutput")

    with tile.TileContext(nc) as tc, ExitStack() as ctx:
        const = ctx.enter_context(tc.tile_pool(name="const", bufs=1))
        sm = ctx.enter_context(tc.tile_pool(name="sm", bufs=1))
        ps_rc = ctx.enter_context(tc.tile_pool(name="ps_rc", bufs=1, space="PSUM"))
        ps_rq = ctx.enter_context(tc.tile_pool(name="ps_rq", bufs=1, space="PSUM"))
        ps_lg = ctx.enter_context(tc.tile_pool(name="ps_lg", bufs=1, space="PSUM"))
        ps_tp = ctx.enter_context(tc.tile_pool(name="ps_tp", bufs=1, space="PSUM"))
        ps_ou = ctx.enter_context(tc.tile_pool(name="ps_ou", bufs=1, space="PSUM"))

        # ---- input DMAs, balanced across the two hardware-DGE queues so
        # res_q (WqT+qT) and res_c (WcT+ctxT chunks) unblock earliest
        WqT_sb = const.tile([P, KD, H], BF16)
        WcT_sb = const.tile([P, KD, H], BF16)
        qT_sb = const.tile([P, KD, Q], BF16)
        ctxT_sb = const.tile([P, KD, C], BF16)
        WoCK_sb = const.tile([P, JH, M_HARM + 2], F32)
        mrow_sb = const.tile([1, C], BF16)
        nc.sync.dma_start(WqT_sb[:], WqT_d.ap().rearrange("(k p) h -> p k h", p=P))
        ctxT_ap = ctxT_d.ap().rearrange("(k p) c -> p k c", p=P)
        nc.sync.dma_start(ctxT_sb[:, 0:2, :], ctxT_ap[:, 0:2, :])
        nc.sync.dma_start(ctxT_sb[:, 2:4, :], ctxT_ap[:, 2:4, :])
        nc.scalar.dma_start(qT_sb[:], qT_d.ap().rearrange("(k p) q -> p k q", p=P))
        nc.scalar.dma_start(WcT_sb[:], WcT_d.ap().rearrange("(k p) h -> p k h", p=P))
        nc.scalar.dma_start(WoCK_sb[:], WoCK_d.ap())
        nc.scalar.dma_start(mrow_sb[:], mrow_d.ap())
        # ctx (only needed by the final weights @ ctx matmul) is triggered
        # after the res matmuls are emitted: readers of any earlier tensor
        # conservatively wait on all previously-issued DMAs
        ctx_sb = const.tile([P, KC, D], BF16)

        onesC = const.tile([P, C], BF16)
        nc.gpsimd.memset(onesC[:], 1.0)
        onesQ = const.tile([P, Q], BF16)
        nc.gpsimd.memset(onesQ[:], 1.0)
        ident = const.tile([Q, Q], F32)
        masks.make_identity(nc, ident[:])
        bo_sb = const.tile([P, 1], F32)
        nc.vector.memset(bo_sb[:], float(b_o_val))

        # ---- PE warmup: junk matmuls while the input DMAs stream, so the
        # tensor engine is at full p-state when res_q starts.  The scratch
        # PSUM bank is fully overwritten by the transposes later.
        warm = const.tile([P, Q], BF16)
        nc.vector.memset(warm[:], 0.25)

        rcp = ps_rc.tile([P, JH, C], F32)
        rqp = ps_rq.tile([P, JH, Q], F32)
        tp = ps_tp.tile([P, KC, Q], F32)
        for i in range(14):
            nc.tensor.matmul(
                tp[0:Q, 0, :], warm[:], warm[:], start=True, stop=True
            )
        # ---- res_q
        for j in range(JH):
            hs = slice(j * P, (j + 1) * P)
            for k in range(KD):
                nc.tensor.matmul(
                    rqp[:, j, :], WqT_sb[:, k, hs], qT_sb[:, k, :],
                    start=(k == 0), stop=(k == KD - 1),
                )
        for i in range(8):  # bridge the ctxT wait, keeping the p-state up
            nc.tensor.matmul(
                tp[0:Q, 1, :], warm[:], warm[:], start=True, stop=True
            )
        # ---- res_c, k-outer so the matmuls chase the two ctxT DMA chunks
        for k in range(KD):
            for j in range(JH):
                hs = slice(j * P, (j + 1) * P)
                nc.tensor.matmul(
                    rcp[:, j, :], WcT_sb[:, k, hs], ctxT_sb[:, k, :],
                    start=(k == 0), stop=(k == KD - 1),
                )
        nc.sync.dma_start(ctx_sb[:], ctx_d.ap().rearrange("(k p) d -> p k d", p=P))

        # ---- B side (query, 64 cols).  rq staging runs on ACT with the
        # b_c fold as a per-partition bias; products and W_o*c_m folds run
        # on Pool so the DVE only carries the wide A-side chain.
        P0 = 2.0 * PI / W0
        rq_sb = sm.tile([P, JH, Q], BF16, name="rq_sb")
        for j in range(JH):
            nc.scalar.activation(
                rq_sb[:, j, :], rqp[:, j, :], Act.Identity,
                bias=WoCK_sb[:, j, M_HARM + 1 : M_HARM + 2], scale=1.0,
            )
        wB = sm.tile([P, JH, Q], F32, name="wB")
        nc.vector.add_range_wrap(wB[:], rq_sb[:], (PI / 2) / W0, P0 / 2, P0)
        sB = sm.tile([P, JH, Q], BF16, name="sB")
        cB = sm.tile([P, JH, Q], BF16, name="cB")
        nc.scalar.activation(sB[:], rq_sb[:], Act.Sin, bias=0.0, scale=W0 * TRIM)
        nc.scalar.activation(cB[:], wB[:], Act.Sin, bias=0.0, scale=W0 * TRIM)

        # linear-term folds (ready first: only need rq_sb / ones)
        BlinQ = sm.tile([P, JH, Q], BF16, name="BlinQ")
        BlinO = sm.tile([P, JH, Q], BF16, name="BlinO")
        for j in range(JH):
            nc.gpsimd.tensor_scalar(
                BlinQ[:, j, :], rq_sb[:, j, :],
                WoCK_sb[:, j, M_HARM : M_HARM + 1], None, AluOpType.mult,
            )
            nc.gpsimd.tensor_scalar(
                BlinO[:, j, :], onesQ[:], WoCK_sb[:, j, M_HARM : M_HARM + 1],
                None, AluOpType.mult,
            )

        # B-side harmonic products (Pool), interleaved with the gB folds
        gB = sm.tile([P, M_HARM, 2, JH, Q], BF16, name="gB")

        def fold(m, t, src):
            for j in range(JH):
                nc.gpsimd.tensor_scalar(
                    gB[:, m, t, j, :], src[:, j, :],
                    WoCK_sb[:, j, m : m + 1], None, AluOpType.mult,
                )

        fold(0, 0, sB)
        fold(0, 1, cB)
        s1sB = sm.tile([P, JH, Q], BF16, name="s1sB")
        nc.gpsimd.tensor_tensor(s1sB[:], sB[:], sB[:], AluOpType.mult)
        s2B = sm.tile([P, JH, Q], BF16, name="s2B")
        nc.gpsimd.tensor_tensor(s2B[:], sB[:], cB[:], AluOpType.mult)
        c2B = sm.tile([P, JH, Q], BF16, name="c2B")
        nc.gpsimd.tensor_scalar(c2B[:], s1sB[:], -2.0, 1.0, AluOpType.mult, AluOpType.add)
        fold(1, 0, s2B)
        fold(1, 1, c2B)
        if M_HARM >= 3:
            c1sB = sm.tile([P, JH, Q], BF16, name="c1sB")
            nc.gpsimd.tensor_tensor(c1sB[:], cB[:], cB[:], AluOpType.mult)
            u3B = sm.tile([P, JH, Q], BF16, name="u3B")
            nc.gpsimd.tensor_scalar(u3B[:], s1sB[:], -4.0, 3.0, AluOpType.mult, AluOpType.add)
            s3B = sm.tile([P, JH, Q], BF16, name="s3B")
            nc.gpsimd.tensor_tensor(s3B[:], u3B[:], sB[:], AluOpType.mult)
            v3B = sm.tile([P, JH, Q], BF16, name="v3B")
            nc.gpsimd.tensor_scalar(v3B[:], c1sB[:], 4.0, -3.0, AluOpType.mult, AluOpType.add)
            c3B = sm.tile([P, JH, Q], BF16, name="c3B")
            nc.gpsimd.tensor_tensor(c3B[:], v3B[:], cB[:], AluOpType.mult)
            fold(2, 0, s3B)
            fold(2, 1, c3B)
        if M_HARM >= 4:
            s2sB = sm.tile([P, JH, Q], BF16, name="s2sB")
            nc.gpsimd.tensor_tensor(s2sB[:], s2B[:], s2B[:], AluOpType.mult)
            s4B = sm.tile([P, JH, Q], BF16, name="s4B")
            nc.gpsimd.tensor_tensor(s4B[:], s2B[:], c2B[:], AluOpType.mult)
            c4B = sm.tile([P, JH, Q], BF16, name="c4B")
            nc.gpsimd.tensor_scalar(c4B[:], s2sB[:], -8.0, 1.0, AluOpType.mult, AluOpType.add)
            fold(3, 0, s4B)
            fold(3, 1, c4B)

        # ---- A side (context, 512 cols): bias-free.  Staging copies and
        # squares on ACT, the wrap and the remaining products on DVE.
        rc_sb = sm.tile([P, JH, C], BF16, name="rc_sb")
        for j in range(JH):
            nc.scalar.activation(rc_sb[:, j, :], rcp[:, j, :], Act.Copy)
        wA = sm.tile([P, JH, C], F32, name="wA")
        nc.vector.add_range_wrap(wA[:], rcp[:], (PI / 2) / W0, P0 / 2, P0)
        sA = sm.tile([P, JH, C], BF16, name="sA")
        cA = sm.tile([P, JH, C], BF16, name="cA")
        nc.scalar.activation(sA[:], rc_sb[:], Act.Sin, bias=0.0, scale=W0 * TRIM)
        nc.scalar.activation(cA[:], wA[:], Act.Sin, bias=0.0, scale=W0 * TRIM)
        s1sA = sm.tile([P, JH, C], BF16, name="s1sA")
        nc.scalar.activation(s1sA[:], sA[:], Act.Square)
        s2A = sm.tile([P, JH, C], BF16, name="s2A")
        nc.vector.tensor_tensor(s2A[:], sA[:], cA[:], AluOpType.mult)
        c2A = sm.tile([P, JH, C], BF16, name="c2A")
        nc.vector.tensor_scalar(c2A[:], s1sA[:], -2.0, 1.0, AluOpType.mult, AluOpType.add)
        if M_HARM >= 3:
            c1sA = sm.tile([P, JH, C], BF16, name="c1sA")
            nc.scalar.activation(c1sA[:], cA[:], Act.Square)
            u3A = sm.tile([P, JH, C], BF16, name="u3A")
            nc.vector.tensor_scalar(u3A[:], s1sA[:], -4.0, 3.0, AluOpType.mult, AluOpType.add)
            s3A = sm.tile([P, JH, C], BF16, name="s3A")
            nc.vector.tensor_tensor(s3A[:], u3A[:], sA[:], AluOpType.mult)
            v3A = sm.tile([P, JH, C], BF16, name="v3A")
            nc.vector.tensor_scalar(v3A[:], c1sA[:], 4.0, -3.0, AluOpType.mult, AluOpType.add)
            c3A = sm.tile([P, JH, C], BF16, name="c3A")
            nc.vector.tensor_tensor(c3A[:], v3A[:], cA[:], AluOpType.mult)
        if M_HARM >= 4:
            s2sA = sm.tile([P, JH, C], BF16, name="s2sA")
            nc.scalar.activation(s2sA[:], s2A[:], Act.Square)
            s4A = sm.tile([P, JH, C], BF16, name="s4A")
            nc.vector.tensor_tensor(s4A[:], s2A[:], c2A[:], AluOpType.mult)
            c4A = sm.tile([P, JH, C], BF16, name="c4A")
            nc.vector.tensor_scalar(c4A[:], s2sA[:], -8.0, 1.0, AluOpType.mult, AluOpType.add)

        # ---- logit contraction [q, c]: one PSUM bank, chunks ordered by
        # feature readiness (linear -> mask -> fundamental -> harmonics)
        lg = ps_lg.tile([Q, C], F32)
        first = dict(v=True)

        def mm(bt, at, stop=False):
            nc.tensor.matmul(lg[:], bt, at, start=first["v"], stop=stop)
            first["v"] = False

        for j in range(JH):
            mm(BlinQ[:, j, :], onesC[:])            # k*Wo.rq' broadcast over c
        mm(onesQ[0:1, :], mrow_sb[:])               # ln(mask) rank-1
        for j in range(JH):
            mm(BlinO[:, j, :], rc_sb[:, j, :])      # k*Wo.rc broadcast over q
        for j in range(JH):
            mm(gB[:, 0, 1, j, :], sA[:, j, :])
            mm(gB[:, 0, 0, j, :], cA[:, j, :])
        FA = [(s2A, c2A)]
        if M_HARM >= 3:
            FA.append((s3A, c3A))
        if M_HARM >= 4:
            FA.append((s4A, c4A))
        for m in range(1, M_HARM):
            fAs, fAc = FA[m - 1]
            last = m == M_HARM - 1
            for j in range(JH):
                mm(gB[:, m, 1, j, :], fAs[:, j, :])
                mm(gB[:, m, 0, j, :], fAc[:, j, :], stop=(last and j == JH - 1))

        # ---- softmax tail: exp (+ masked row sums via accum_out), then the
        # unnormalized exp is PE-transposed against diag(1/rowsum) so the
        # final weights @ ctx matmul emits normalized output directly
        expQ = sm.tile([Q, C], F32)
        sumQ = sm.tile([Q, 1], F32)
        nc.scalar.activation(
            expQ[:], lg[:], Act.Exp, bias=bo_sb[0:Q, 0:1], accum_out=sumQ[:]
        )
        for i in range(5):  # keep the PE p-state up through the exp wait
            nc.tensor.matmul(
                tp[0:Q, 0, :], warm[:], warm[:], start=True, stop=True
            )
        recQ = sm.tile([Q, 1], F32)
        nc.vector.tensor_scalar_add(recQ[:], sumQ[:], float(EPS))
        nc.vector.reciprocal(recQ[:], recQ[:])
        diag = sm.tile([Q, Q], F32)
        nc.gpsimd.tensor_scalar(
            diag[:], ident[:], recQ[:, 0:1], None, AluOpType.mult
        )
        # transpose-and-normalize: a regular fp32 matmul expQ_chunk.T @ diag
        # (diag is NOT a permutation, so PE transpose mode can't be used)
        for k in range(KC):
            nc.tensor.matmul(
                tp[:, k, :], expQ[:, k * P : (k + 1) * P], diag[:],
                start=True, stop=True,
            )
        w_sb = sm.tile([Q, C], F32)
        nc.vector.tensor_scalar(
            w_sb[:], expQ[:], recQ[:, 0:1], None, AluOpType.mult
        )
        nc.scalar.dma_start(wts_d.ap()[:, :], w_sb[:])
        eT_sb = sm.tile([P, KC, Q], BF16)
        nc.scalar.activation(eT_sb[:], tp[:], Act.Copy)
        ou = ps_ou.tile([Q, D], F32)
        for i in range(3):  # bridge the eT staging wait on a warm PE
            nc.tensor.matmul(
                tp[0:Q, 1, :], warm[:], warm[:], start=True, stop=True
            )
        for k in range(KC):
            nc.tensor.matmul(
                ou[:], eT_sb[:, k, :], ctx_sb[:, k, :],
                start=(k == 0), stop=(k == KC - 1),
            )
        out_sb = sm.tile([Q, D], F32)
        nc.scalar.activation(out_sb[:], ou[:], Act.Copy)
        nc.sync.dma_start(out_d.ap()[:, :], out_sb[:])

    nc.compile()
    return nc


def make_in_maps(query, context, mask, W_c, b_c, W_q, W_o):
    import ml_dtypes
    f32 = np.float32
    bf16 = ml_dtypes.bfloat16
    WqT = np.ascontiguousarray(np.asarray(W_q, f32).T.astype(bf16))
    WcT = np.ascontiguousarray(np.asarray(W_c, f32).T.astype(bf16))
    Wo2 = np.asarray(W_o, f32).reshape(JH, P).T  # (P, JH)
    # the m=2 product feature is sin2/2, so its fold carries 2x
    cols = [f32(c) for c in CS]
    cols[1] = f32(2.0) * cols[1]
    if M_HARM >= 4:
        cols[3] = f32(4.0) * cols[3]
    cols.append(f32(K_LIN))
    WoCK = np.stack([Wo2 * c for c in cols], axis=2)  # (P, JH, M+1)
    bc2 = np.asarray(b_c, f32).reshape(JH, P).T[:, :, None]  # (P, JH, 1)
    WoCKB = np.ascontiguousarray(
        np.concatenate([WoCK, bc2], axis=2).astype(f32)
    )  # (P, JH, M+2)
    in_maps = []
    for b in range(B):
        mrow = np.asarray(mask[b], f32)
        mbr = np.maximum(np.log(np.maximum(mrow, 1e-300)), -50.0)
        in_maps.append(
            {
                "WqT": WqT,
                "WcT": WcT,
                "qT": np.ascontiguousarray(np.asarray(query[b], f32).T.astype(bf16)),
                "ctxT": np.ascontiguousarray(
                    np.asarray(context[b], f32).T.astype(bf16)
                ),
                "ctx": np.ascontiguousarray(np.asarray(context[b], bf16)),
                "mrow": np.ascontiguousarray(mbr.reshape(1, C).astype(bf16)),
                "WoCK": WoCKB,
            }
        )
    return in_maps


def kernel(query, context, mask, W_c, b_c, W_q, W_o, b_o):
    from concourse.bass_utils import run_bass_kernel_spmd

    nc = _build_program(float(np.asarray(b_o)))
    in_maps = make_in_maps(query, context, mask, W_c, b_c, W_q, W_o)
    res = run_bass_kernel_spmd(nc, in_maps, list(range(N_CORES))).results
    out = np.stack([res[b]["out"] for b in range(B)])
    wts = np.stack([res[b]["wts"] for b in range(B)])
    return out, wts


# revision 5
# speedup vs baseline: 1.4170x; 1.4170x over previous
"""Bass/Tile TRN2 kernel for nn_Attention (additive/Bahdanau-style attention).

reference math per batch b:
  res_q = query[b] @ W_q.T                      (Q, H)
  res_c = context[b] @ W_c.T + b_c              (C, H)
  logit[q,c] = sum_h W_o[h]*tanh(res_c[c,h] + res_q[q,h]) + b_o
  w = mask * exp(logit); weights = w / (sum_c w + eps)
  out = weights @ context[b]

The (Q,C,H) tanh grid is never materialized. tanh is replaced by an
M-term harmonic sine series  tanh(x) ~= k*x + sum_m c_m sin(m*w0*x), and
the angle-addition identity factorizes each term so the whole logit is
ONE PE contraction over (h, m, trig) chunks of 128:
  logit[q,c] = sum_f B_f[h,q] * A_f[h,c]
b_c folds into the rq staging copy (ACT Identity with per-partition
bias), so the A(context) side is bias-free.  Only the fundamental
sin/cos touch the ACT Sin table (cos through one DVE add_range_wrap
with the pi/2 phase folded into the wrap shift); harmonics 2..3 come
from double/triple angle products, with the squares on the (otherwise
idle) ACT engine and everything else on the DVE.  The 2x factor of
s2'=sin2/2 is folded into the host-side W_o*c_m coefficients.  A rank-1
ln(mask) chunk folds the mask into the logit so exp's accum_out
directly yields the masked softmax denominator.  The PE "transpose" of
the exp is a plain fp32 matmul against diag(1/rowsum), so the final
weights @ ctx matmul emits already-normalized output and the whole
epilogue has no wide DVE op.

All matmul operands are bf16; PSUM accumulation stays f32.  Every bulk
input is pre-rearranged on the host into a [128, n*cols] partition-major
layout so each DMA moves few large contiguous lines; bulk rides the
sync hardware-DGE queue (the scalar-triggered queue is much slower),
ordered so res_q and then res_c unblock earliest.

Sharding: data-parallel over batch B=8 across the 8 NeuronCores.
"""

import numpy as np

B, Q, C, D, H = 8, 64, 512, 512, 256
EPS = 1e-5
P = 128
KD = D // P   # 4 chunks of the contraction dim d
KC = C // P   # 4 chunks of the context dim c
JH = H // P   # 2 chunks of the hidden dim h
N_CORES = 8

# tanh(x) ~= K_LIN*x + sum_m CS[m]*sin(m*W0*x), fitted on [-4.9, 4.9]
M_HARM = 3
W0 = 0.90
K_LIN = 0.2846621628161513
CS = [0.4703299122559558, 0.1052940908727595, 0.02379076660069648]
PI = float(np.pi)
TRIM = 1.0 - 5e-7    # keeps |w0*x| strictly inside the Sin table domain


def _build_program(b_o_val: float):
    import concourse.bacc as bacc
    import concourse.mybir as mybir
    import concourse.tile as tile
    from concourse.alu_op_type import AluOpType
    from concourse import masks
    from contextlib import ExitStack

    F32 = mybir.dt.float32
    BF16 = mybir.dt.bfloat16
    Act = mybir.ActivationFunctionType

    nc = bacc.Bacc("TRN2", target_bir_lowering=False, debug=False)

    # all bulk inputs arrive pre-rearranged: [P, k*cols], partition-major
    WqT_d = nc.dram_tensor("WqT", [P, KD * H], BF16, kind="ExternalInput")
    WcT_d = nc.dram_tensor("WcT", [P, KD * H], BF16, kind="ExternalInput")
    qT_d = nc.dram_tensor("qT", [P, KD * Q], BF16, kind="ExternalInput")
    ctxT_d = nc.dram_tensor("ctxT", [P, KD * C], BF16, kind="ExternalInput")
    ctx_d = nc.dram_tensor("ctx", [P, KC * D], BF16, kind="ExternalInput")
    mrow_d = nc.dram_tensor("mrow", [1, C], BF16, kind="ExternalInput")
    # cols 0..M-1: W_o * c_m folds (2x on m=2 for the halved sin2 product);
    # col M: W_o * k_lin; col M+1: b_c
    WoCK_d = nc.dram_tensor("WoCK", [P, JH, M_HARM + 2], F32, kind="ExternalInput")
    out_d = nc.dram_tensor("out", [Q, D], F32, kind="ExternalOutput")
    wts_d = nc.dram_tensor("wts", [Q, C], F32, kind="ExternalOutput")

    with tile.TileContext(nc) as tc, ExitStack() as ctx:
        const = ctx.enter_context(tc.tile_pool(name="const", bufs=1))
        sm = ctx.enter_context(tc.tile_pool(name="sm", bufs=1))
        ps_rc = ctx.enter_context(tc.tile_pool(name="ps_rc", bufs=1, space="PSUM"))
        ps_rq = ctx.enter_context(tc.tile_pool(name="ps_rq", bufs=1, space="PSUM"))
        ps_lg = ctx.enter_context(tc.tile_pool(name="ps_lg", bufs=1, space="PSUM"))
        ps_tp = ctx.enter_context(tc.tile_pool(name="ps_tp", bufs=1, space="PSUM"))
        ps_ou = ctx.enter_context(tc.tile_pool(name="ps_ou", bufs=1, space="PSUM"))

        # ---- input DMAs.  Bulk on the sync queue in dependency order;
        # smalls on the (slow) scalar queue.
        WqT_sb = const.tile([P, KD, H], BF16)
        WcT_sb = const.tile([P, KD, H], BF16)
        qT_sb = const.tile([P, KD, Q], BF16)
        ctxT_sb = const.tile([P, KD, C], BF16)
        WoCK_sb = const.tile([P, JH, M_HARM + 2], F32)
        mrow_sb = const.tile([1, C], BF16)
        nc.sync.dma_start(WqT_sb[:], WqT_d.ap())
        nc.sync.dma_start(WcT_sb[:], WcT_d.ap())
        nc.sync.dma_start(ctxT_sb[:, 0:2, :], ctxT_d.ap()[:, 0 : 2 * C])
        nc.sync.dma_start(ctxT_sb[:, 2:4, :], ctxT_d.ap()[:, 2 * C : 4 * C])
        nc.scalar.dma_start(qT_sb[:], qT_d.ap())
        nc.scalar.dma_start(WoCK_sb[:], WoCK_d.ap())
        nc.scalar.dma_start(mrow_sb[:], mrow_d.ap())
        # ctx (only needed by the final weights @ ctx matmul) is triggered
        # after the res matmuls are emitted: readers of any earlier tensor
        # conservatively wait on all previously-issued DMAs
        ctx_sb = const.tile([P, KC, D], BF16)

        onesC = const.tile([P, C], BF16)
        nc.gpsimd.memset(onesC[:], 1.0)
        onesQ = const.tile([P, Q], BF16)
        nc.gpsimd.memset(onesQ[:], 1.0)
        ident = const.tile([Q, Q], F32)
        masks.make_identity(nc, ident[:])
        bo_sb = const.tile([P, 1], F32)
        nc.vector.memset(bo_sb[:], float(b_o_val))
        # dummy first ACT op: forces the trig table (which also contains
        # Identity/Copy/Square) to be the one loaded during the DMA stream
        warm = const.tile([P, Q], BF16)
        nc.vector.memset(warm[:], 0.25)
        sinwarm = sm.tile([1, 1], BF16, name="sinwarm")
        nc.scalar.activation(sinwarm[:], warm[0:1, 0:1], Act.Sin, bias=0.0, scale=W0)

        # ---- PE warmup: junk matmuls while the input DMAs stream, so the
        # tensor engine is at full p-state when res_q starts.  The scratch
        # PSUM bank is fully overwritten by the transposes later.
        rcp = ps_rc.tile([P, JH, C], F32)
        rqp = ps_rq.tile([P, JH, Q], F32)
        tp = ps_tp.tile([P, KC, Q], F32)
        for i in range(16):
            nc.tensor.matmul(
                tp[0:Q, 0, :], warm[:], warm[:], start=True, stop=True
            )
        # ---- res_q
        for j in range(JH):
            hs = slice(j * P, (j + 1) * P)
            for k in range(KD):
                nc.tensor.matmul(
                    rqp[:, j, :], WqT_sb[:, k, hs], qT_sb[:, k, :],
                    start=(k == 0), stop=(k == KD - 1),
                )
        for i in range(6):  # bridge the ctxT wait, keeping the p-state up
            nc.tensor.matmul(
                tp[0:Q, 1, :], warm[:], warm[:], start=True, stop=True
            )
        # ---- res_c, k-outer so the matmuls chase the two ctxT DMA chunks
        for k in range(KD):
            for j in range(JH):
                hs = slice(j * P, (j + 1) * P)
                nc.tensor.matmul(
                    rcp[:, j, :], WcT_sb[:, k, hs], ctxT_sb[:, k, :],
                    start=(k == 0), stop=(k == KD - 1),
                )
        nc.sync.dma_start(ctx_sb[:], ctx_d.ap())

        # ---- B side (query, 64 cols): rq staging on ACT with the b_c fold
        # as a per-partition bias, then products + W_o*c_m folds on DVE.
        P0 = 2.0 * PI / W0
        rq_sb = sm.tile([P, JH, Q], BF16, name="rq_sb")
        for j in range(JH):
            nc.scalar.activation(
                rq_sb[:, j, :], rqp[:, j, :], Act.Identity,
                bias=WoCK_sb[:, j, M_HARM + 1 : M_HARM + 2], scale=1.0,
            )
        wB = sm.tile([P, JH, Q], F32, name="wB")
        nc.vector.add_range_wrap(wB[:], rq_sb[:], (PI / 2) / W0, P0 / 2, P0)
        sB = sm.tile([P, JH, Q], BF16, name="sB")
        cB = sm.tile([P, JH, Q], BF16, name="cB")
        nc.scalar.activation(sB[:], rq_sb[:], Act.Sin, bias=0.0, scale=W0 * TRIM)
        nc.scalar.activation(cB[:], wB[:], Act.Sin, bias=0.0, scale=W0 * TRIM)

        # linear-term folds (ready first: only need rq_sb / ones)
        BlinQ = sm.tile([P, JH, Q], BF16, name="BlinQ")
        BlinO = sm.tile([P, JH, Q], BF16, name="BlinO")
        for j in range(JH):
            nc.vector.tensor_scalar(
                BlinQ[:, j, :], rq_sb[:, j, :],
                WoCK_sb[:, j, M_HARM : M_HARM + 1], None, AluOpType.mult,
            )
            nc.vector.tensor_scalar(
                BlinO[:, j, :], onesQ[:], WoCK_sb[:, j, M_HARM : M_HARM + 1],
                None, AluOpType.mult,
            )

        # B-side harmonic products + folds (DVE; these are small)
        gB = sm.tile([P, M_HARM, 2, JH, Q], BF16, name="gB")

        def fold(m, t, src):
            for j in range(JH):
                nc.vector.tensor_scalar(
                    gB[:, m, t, j, :], src[:, j, :],
                    WoCK_sb[:, j, m : m + 1], None, AluOpType.mult,
                )

        fold(0, 0, sB)
        fold(0, 1, cB)
        s1sB = sm.tile([P, JH, Q], BF16, name="s1sB")
        nc.vector.tensor_tensor(s1sB[:], sB[:], sB[:], AluOpType.mult)
        s2B = sm.tile([P, JH, Q], BF16, name="s2B")
        nc.vector.tensor_tensor(s2B[:], sB[:], cB[:], AluOpType.mult)
        c2B = sm.tile([P, JH, Q], BF16, name="c2B")
        nc.vector.tensor_scalar(c2B[:], s1sB[:], -2.0, 1.0, AluOpType.mult, AluOpType.add)
        fold(1, 0, s2B)
        fold(1, 1, c2B)
        if M_HARM >= 3:
            c1sB = sm.tile([P, JH, Q], BF16, name="c1sB")
            nc.vector.tensor_tensor(c1sB[:], cB[:], cB[:], AluOpType.mult)
            u3B = sm.tile([P, JH, Q], BF16, name="u3B")
            nc.vector.tensor_scalar(u3B[:], s1sB[:], -4.0, 3.0, AluOpType.mult, AluOpType.add)
            s3B = sm.tile([P, JH, Q], BF16, name="s3B")
            nc.vector.tensor_tensor(s3B[:], u3B[:], sB[:], AluOpType.mult)
            v3B = sm.tile([P, JH, Q], BF16, name="v3B")
            nc.vector.tensor_scalar(v3B[:], c1sB[:], 4.0, -3.0, AluOpType.mult, AluOpType.add)
            c3B = sm.tile([P, JH, Q], BF16, name="c3B")
            nc.vector.tensor_tensor(c3B[:], v3B[:], cB[:], AluOpType.mult)
            fold(2, 0, s3B)
            fold(2, 1, c3B)
        if M_HARM >= 4:
            s2sB = sm.tile([P, JH, Q], BF16, name="s2sB")
            nc.vector.tensor_tensor(s2sB[:], s2B[:], s2B[:], AluOpType.mult)
            s4B = sm.tile([P, JH, Q], BF16, name="s4B")
            nc.vector.tensor_tensor(s4B[:], s2B[:], c2B[:], AluOpType.mult)
            c4B = sm.tile([P, JH, Q], BF16, name="c4B")
            nc.vector.tensor_scalar(c4B[:], s2sB[:], -8.0, 1.0, AluOpType.mult, AluOpType.add)
            fold(3, 0, s4B)
            fold(3, 1, c4B)

        # ---- A side (context, 512 cols): bias-free.  Staging copies and
        # squares on ACT, the wrap and the remaining products on DVE.
        rc_sb = sm.tile([P, JH, C], BF16, name="rc_sb")
        for j in range(JH):
            nc.scalar.activation(rc_sb[:, j, :], rcp[:, j, :], Act.Copy)
        wA = sm.tile([P, JH, C], F32, name="wA")
        nc.vector.add_range_wrap(wA[:], rcp[:], (PI / 2) / W0, P0 / 2, P0)
        sA = sm.tile([P, JH, C], BF16, name="sA")
        cA = sm.tile([P, JH, C], BF16, name="cA")
        nc.scalar.activation(sA[:], rc_sb[:], Act.Sin, bias=0.0, scale=W0 * TRIM)
        nc.scalar.activation(cA[:], wA[:], Act.Sin, bias=0.0, scale=W0 * TRIM)
        s1sA = sm.tile([P, JH, C], BF16, name="s1sA")
        nc.scalar.activation(s1sA[:], sA[:], Act.Square)
        s2A = sm.tile([P, JH, C], BF16, name="s2A")
        nc.vector.tensor_tensor(s2A[:], sA[:], cA[:], AluOpType.mult)
        c2A = sm.tile([P, JH, C], BF16, name="c2A")
        nc.vector.tensor_scalar(c2A[:], s1sA[:], -2.0, 1.0, AluOpType.mult, AluOpType.add)
        if M_HARM >= 3:
            c1sA = sm.tile([P, JH, C], BF16, name="c1sA")
            nc.scalar.activation(c1sA[:], cA[:], Act.Square)
            u3A = sm.tile([P, JH, C], BF16, name="u3A")
            nc.vector.tensor_scalar(u3A[:], s1sA[:], -4.0, 3.0, AluOpType.mult, AluOpType.add)
            s3A = sm.tile([P, JH, C], BF16, name="s3A")
            nc.vector.tensor_tensor(s3A[:], u3A[:], sA[:], AluOpType.mult)
            v3A = sm.tile([P, JH, C], BF16, name="v3A")
            nc.vector.tensor_scalar(v3A[:], c1sA[:], 4.0, -3.0, AluOpType.mult, AluOpType.add)
            c3A = sm.tile([P, JH, C], BF16, name="c3A")
            nc.vector.tensor_tensor(c3A[:], v3A[:], cA[:], AluOpType.mult)
        if M_HARM >= 4:
            s2sA = sm.tile([P, JH, C], BF16, name="s2sA")
            nc.scalar.activation(s2sA[:], s2A[:], Act.Square)
            s4A = sm.tile([P, JH, C], BF16, name="s4A")
            nc.vector.tensor_tensor(s4A[:], s2A[:], c2A[:], AluOpType.mult)
            c4A = sm.tile([P, JH, C], BF16, name="c4A")
            nc.vector.tensor_scalar(c4A[:], s2sA[:], -8.0, 1.0, AluOpType.mult, AluOpType.add)

        # ---- logit contraction [q, c]: one PSUM bank, chunks ordered by
        # feature readiness (linear -> mask -> fundamental -> harmonics)
        lg = ps_lg.tile([Q, C], F32)
        first = dict(v=True)

        def mm(bt, at, stop=False):
            nc.tensor.matmul(lg[:], bt, at, start=first["v"], stop=stop)
            first["v"] = False

        for j in range(JH):
            mm(BlinQ[:, j, :], onesC[:])            # k*Wo.rq' broadcast over c
        mm(onesQ[0:1, :], mrow_sb[:])               # ln(mask) rank-1
        for j in range(JH):
            mm(BlinO[:, j, :], rc_sb[:, j, :])      # k*Wo.rc broadcast over q
        for j in range(JH):
            mm(gB[:, 0, 1, j, :], sA[:, j, :])
            mm(gB[:, 0, 0, j, :], cA[:, j, :])
        FA = [(s2A, c2A)]
        if M_HARM >= 3:
            FA.append((s3A, c3A))
        if M_HARM >= 4:
            FA.append((s4A, c4A))
        for m in range(1, M_HARM):
            fAs, fAc = FA[m - 1]
            last = m == M_HARM - 1
            for j in range(JH):
                mm(gB[:, m, 1, j, :], fAs[:, j, :])
                mm(gB[:, m, 0, j, :], fAc[:, j, :], stop=(last and j == JH - 1))

        # ---- softmax tail: exp (+ masked row sums via accum_out); the
        # "transpose" of the exp is a plain fp32 matmul against
        # diag(1/rowsum), so weights @ ctx emits normalized output directly
        expQ = sm.tile([Q, C], F32)
        sumQ = sm.tile([Q, 1], F32)
        nc.scalar.activation(
            expQ[:], lg[:], Act.Exp, bias=bo_sb[0:Q, 0:1], accum_out=sumQ[:]
        )
        for i in range(5):  # keep the PE p-state up through the exp wait
            nc.tensor.matmul(
                tp[0:Q, 0, :], warm[:], warm[:], start=True, stop=True
            )
        recQ = sm.tile([Q, 1], F32)
        nc.vector.tensor_scalar_add(recQ[:], sumQ[:], float(EPS))
        nc.vector.reciprocal(recQ[:], recQ[:])
        diag = sm.tile([Q, Q], F32)
        nc.vector.tensor_scalar(
            diag[:], ident[:], recQ[:, 0:1], None, AluOpType.mult
        )
        for k in range(KC):
            nc.tensor.matmul(
                tp[:, k, :], expQ[:, k * P : (k + 1) * P], diag[:],
                start=True, stop=True,
            )
        w_sb = sm.tile([Q, C], F32)
        nc.vector.tensor_scalar(
            w_sb[:], expQ[:], recQ[:, 0:1], None, AluOpType.mult
        )
        nc.scalar.dma_start(wts_d.ap()[:, :], w_sb[:])
        eT_sb = sm.tile([P, KC, Q], BF16)
        nc.scalar.activation(eT_sb[:], tp[:], Act.Copy)
        ou = ps_ou.tile([Q, D], F32)
        for i in range(3):  # bridge the eT staging wait on a warm PE
            nc.tensor.matmul(
                tp[0:Q, 1, :], warm[:], warm[:], start=True, stop=True
            )
        for k in range(KC):
            nc.tensor.matmul(
                ou[:], eT_sb[:, k, :], ctx_sb[:, k, :],
                start=(k == 0), stop=(k == KC - 1),
            )
        out_sb = sm.tile([Q, D], F32)
        nc.scalar.activation(out_sb[:], ou[:], Act.Copy)
        nc.sync.dma_start(out_d.ap()[:, :], out_sb[:])

    nc.compile()
    return nc


def _chunked(a, p=P):
    """[N*p, cols] -> [p, N*cols]: row k*p+i lands at [i, k*cols:(k+1)*cols]."""
    n = a.shape[0] // p
    return np.ascontiguousarray(
        a.reshape(n, p, a.shape[1]).transpose(1, 0, 2).reshape(p, n * a.shape[1])
    )


def make_in_maps(query, context, mask, W_c, b_c, W_q, W_o):
    import ml_dtypes
    f32 = np.float32
    bf16 = ml_dtypes.bfloat16
    WqT = _chunked(np.asarray(W_q, f32).T.astype(bf16))
    WcT = _chunked(np.asarray(W_c, f32).T.astype(bf16))
    Wo2 = np.asarray(W_o, f32).reshape(JH, P).T  # (P, JH)
    # the m=2 product feature is sin2/2, so its fold carries 2x
    cols = [f32(c) for c in CS]
    cols[1] = f32(2.0) * cols[1]
    if M_HARM >= 4:
        cols[3] = f32(4.0) * cols[3]
    cols.append(f32(K_LIN))
    WoCK = np.stack([Wo2 * c for c in cols], axis=2)  # (P, JH, M+1)
    bc2 = np.asarray(b_c, f32).reshape(JH, P).T[:, :, None]  # (P, JH, 1)
    WoCKB = np.ascontiguousarray(
        np.concatenate([WoCK, bc2], axis=2).astype(f32)
    )  # (P, JH, M+2)
    in_maps = []
    for b in range(B):
        mrow = np.asarray(mask[b], f32)
        mbr = np.maximum(np.log(np.maximum(mrow, 1e-300)), -50.0)
        in_maps.append(
            {
                "WqT": WqT,
                "WcT": WcT,
                "qT": _chunked(np.asarray(query[b], f32).T.astype(bf16)),
                "ctxT": _chunked(np.asarray(context[b], f32).T.astype(bf16)),
                "ctx": _chunked(np.asarray(context[b], bf16)),
                "mrow": np.ascontiguousarray(mbr.reshape(1, C).astype(bf16)),
                "WoCK": WoCKB,
            }
        )
    return in_maps


def kernel(query, context, mask, W_c, b_c, W_q, W_o, b_o):
    from concourse.bass_utils import run_bass_kernel_spmd

    nc = _build_program(float(np.asarray(b_o)))
    in_maps = make_in_maps(query, context, mask, W_c, b_c, W_q, W_o)
    res = run_bass_kernel_spmd(nc, in_maps, list(range(N_CORES))).results
    out = np.stack([res[b]["out"] for b in range(B)])
    wts = np.stack([res[b]["wts"] for b in range(B)])
    return out, wts


# revision 10
# speedup vs baseline: 1.5665x; 1.1055x over previous
"""Bass/Tile TRN2 kernel for nn_Attention (additive/Bahdanau-style attention).

reference math per batch b:
  res_q = query[b] @ W_q.T                      (Q, H)
  res_c = context[b] @ W_c.T + b_c              (C, H)
  logit[q,c] = sum_h W_o[h]*tanh(res_c[c,h] + res_q[q,h]) + b_o
  w = mask * exp(logit); weights = w / (sum_c w + eps)
  out = weights @ context[b]

The (Q,C,H) tanh grid is never materialized. tanh is replaced by an
M-term harmonic sine series  tanh(x) ~= k*x + sum_m c_m sin(m*w0*x), and
the angle-addition identity factorizes each term so the whole logit is
ONE PE contraction over (h, m, trig) chunks of 128:
  logit[q,c] = sum_f B_f[h,q] * A_f[h,c]
b_c folds into the rq staging copy (ACT Identity with per-partition
bias), so the A(context) side is bias-free.  Only the fundamental
sin/cos touch the ACT Sin table (cos through one DVE add_range_wrap
with the pi/2 phase folded into the wrap shift); harmonics 2..3 come
from double/triple angle products, with the squares on the (otherwise
idle) ACT engine and everything else on the DVE.  The 2x factor of
s2'=sin2/2 is folded into the host-side W_o*c_m coefficients.  A rank-1
ln(mask) chunk folds the mask into the logit so exp's accum_out
directly yields the masked softmax denominator.  The PE "transpose" of
the exp is a plain fp32 matmul against diag(1/rowsum), so the final
weights @ ctx matmul emits already-normalized output and the whole
epilogue has no wide DVE op.

All matmul operands are bf16; PSUM accumulation stays f32.  Every bulk
input is pre-rearranged on the host into a [128, n*cols] partition-major
layout so each DMA moves few large contiguous lines; bulk rides the
sync hardware-DGE queue (the scalar-triggered queue is much slower),
ordered so res_q and then res_c unblock earliest.

Sharding: data-parallel over batch B=8 across the 8 NeuronCores.
"""

import numpy as np

B, Q, C, D, H = 8, 64, 512, 512, 256
EPS = 1e-5
P = 128
KD = D // P   # 4 chunks of the contraction dim d
KC = C // P   # 4 chunks of the context dim c
JH = H // P   # 2 chunks of the hidden dim h
N_CORES = 8

# tanh(x) ~= K_LIN*x + sum_m CS[m]*sin(m*W0*x), fitted on [-4.9, 4.9]
M_HARM = 3
W0 = 0.90
K_LIN = 0.2846621628161513
CS = [0.4703299122559558, 0.1052940908727595, 0.02379076660069648]
PI = float(np.pi)
TRIM = 1.0 - 5e-7    # keeps |w0*x| strictly inside the Sin table domain


def _build_program(b_o_val: float):
    import concourse.bacc as bacc
    import concourse.mybir as mybir
    import concourse.tile as tile
    from concourse.alu_op_type import AluOpType
    from concourse import masks
    from contextlib import ExitStack

    F32 = mybir.dt.float32
    BF16 = mybir.dt.bfloat16
    Act = mybir.ActivationFunctionType

    nc = bacc.Bacc("TRN2", target_bir_lowering=False, debug=False)

    # all bulk inputs arrive pre-rearranged: [P, k*cols], partition-major
    WqT_d = nc.dram_tensor("WqT", [P, KD * H], BF16, kind="ExternalInput")
    WcT_d = nc.dram_tensor("WcT", [P, KD * H], BF16, kind="ExternalInput")
    qT_d = nc.dram_tensor("qT", [P, KD * Q], BF16, kind="ExternalInput")
    ctxT_d = nc.dram_tensor("ctxT", [P, KD * C], BF16, kind="ExternalInput")
    ctx_d = nc.dram_tensor("ctx", [P, KC * D], BF16, kind="ExternalInput")
    mrow_d = nc.dram_tensor("mrow", [1, C], BF16, kind="ExternalInput")
    # cols 0..M-1: W_o * c_m folds (2x on m=2 for the halved sin2 product);
    # col M: W_o * k_lin; col M+1: b_c
    WoCK_d = nc.dram_tensor("WoCK", [P, JH, M_HARM + 2], F32, kind="ExternalInput")
    out_d = nc.dram_tensor("out", [Q, D], F32, kind="ExternalOutput")
    wts_d = nc.dram_tensor("wts", [Q, C], F32, kind="ExternalOutput")

    with tile.TileContext(nc) as tc, ExitStack() as ctx:
        const = ctx.enter_context(tc.tile_pool(name="const", bufs=1))
        sm = ctx.enter_context(tc.tile_pool(name="sm", bufs=1))
        ps_rc = ctx.enter_context(tc.tile_pool(name="ps_rc", bufs=1, space="PSUM"))
        ps_rq = ctx.enter_context(tc.tile_pool(name="ps_rq", bufs=1, space="PSUM"))
        ps_lg = ctx.enter_context(tc.tile_pool(name="ps_lg", bufs=1, space="PSUM"))
        ps_tp = ctx.enter_context(tc.tile_pool(name="ps_tp", bufs=1, space="PSUM"))
        ps_ou = ctx.enter_context(tc.tile_pool(name="ps_ou", bufs=1, space="PSUM"))

        # ---- input DMAs.  Bulk on the sync queue in dependency order;
        # smalls on the (slow) scalar queue.
        WqT_sb = const.tile([P, KD, H], BF16)
        WcT_sb = const.tile([P, KD, H], BF16)
        qT_sb = const.tile([P, KD, Q], BF16)
        ctxT_sb = const.tile([P, KD, C], BF16)
        WoCK_sb = const.tile([P, JH, M_HARM + 2], F32)
        mrow_sb = const.tile([1, C], BF16)
        # both HW-DGE queues share one DMA engine/AXI port (~200GB/s), so
        # bulk goes on sync in dependency order; scalar takes only tinies
        nc.sync.dma_start(qT_sb[:], qT_d.ap())
        nc.sync.dma_start(WqT_sb[:], WqT_d.ap())
        nc.sync.dma_start(WcT_sb[:], WcT_d.ap())
        nc.sync.dma_start(ctxT_sb[:, 0:2, :], ctxT_d.ap()[:, 0 : 2 * C])
        nc.sync.dma_start(ctxT_sb[:, 2:4, :], ctxT_d.ap()[:, 2 * C : 4 * C])
        nc.scalar.dma_start(WoCK_sb[:], WoCK_d.ap())
        nc.scalar.dma_start(mrow_sb[:], mrow_d.ap())
        # ctx (only needed by the final weights @ ctx matmul) is triggered
        # after the res matmuls are emitted: readers of any earlier tensor
        # conservatively wait on all previously-issued DMAs
        ctx_sb = const.tile([P, KC, D], BF16)

        onesC = const.tile([P, C], BF16)
        nc.gpsimd.memset(onesC[:], 1.0)
        onesQ = const.tile([P, Q], BF16)
        nc.gpsimd.memset(onesQ[:], 1.0)
        ident = const.tile([Q, Q], F32)
        masks.make_identity(nc, ident[:])
        bo_sb = const.tile([P, 1], F32)
        nc.vector.memset(bo_sb[:], float(b_o_val))
        # dummy first ACT op: forces the trig table (which also contains
        # Identity/Copy/Square) to be the one loaded during the DMA stream
        warm = const.tile([P, Q], BF16)
        nc.vector.memset(warm[:], 0.25)
        sinwarm = sm.tile([1, 1], BF16, name="sinwarm")
        nc.scalar.activation(sinwarm[:], warm[0:1, 0:1], Act.Sin, bias=0.0, scale=W0)

        # ---- PE warmup: junk matmuls while the input DMAs stream, so the
        # tensor engine is at full p-state when res_q starts.  The scratch
        # PSUM bank is fully overwritten by the transposes later.
        rcp = ps_rc.tile([P, JH, C], F32)
        rqp = ps_rq.tile([P, JH, Q], F32)
        tp = ps_tp.tile([P, KC, Q], F32)
        for i in range(30):
            nc.tensor.matmul(
                tp[0:Q, 0, :], warm[:], warm[:], start=True, stop=True
            )
        # ---- res_q
        for j in range(JH):
            hs = slice(j * P, (j + 1) * P)
            for k in range(KD):
                nc.tensor.matmul(
                    rqp[:, j, :], WqT_sb[:, k, hs], qT_sb[:, k, :],
                    start=(k == 0), stop=(k == KD - 1),
                )
        for i in range(22):  # bridge the ctxT wait, keeping the p-state up
            nc.tensor.matmul(
                tp[0:Q, 1, :], warm[:], warm[:], start=True, stop=True
            )
        # ---- res_c, k-outer so the matmuls chase the two ctxT DMA chunks
        for k in range(KD):
            for j in range(JH):
                hs = slice(j * P, (j + 1) * P)
                nc.tensor.matmul(
                    rcp[:, j, :], WcT_sb[:, k, hs], ctxT_sb[:, k, :],
                    start=(k == 0), stop=(k == KD - 1),
                )
        nc.sync.dma_start(ctx_sb[:], ctx_d.ap())

        # ---- B side (query, 64 cols): rq staging on ACT with the b_c fold
        # as a per-partition bias, then products + W_o*c_m folds on DVE.
        P0 = 2.0 * PI / W0
        rq_sb = sm.tile([P, JH, Q], BF16, name="rq_sb")
        for j in range(JH):
            nc.scalar.activation(
                rq_sb[:, j, :], rqp[:, j, :], Act.Identity,
                bias=WoCK_sb[:, j, M_HARM + 1 : M_HARM + 2], scale=1.0,
            )
        wB = sm.tile([P, JH, Q], F32, name="wB")
        nc.vector.add_range_wrap(wB[:], rq_sb[:], (PI / 2) / W0, P0 / 2, P0)
        sB = sm.tile([P, JH, Q], BF16, name="sB")
        cB = sm.tile([P, JH, Q], BF16, name="cB")
        nc.scalar.activation(sB[:], rq_sb[:], Act.Sin, bias=0.0, scale=W0 * TRIM)
        nc.scalar.activation(cB[:], wB[:], Act.Sin, bias=0.0, scale=W0 * TRIM)

        # linear-term folds (ready first: only need rq_sb / ones)
        BlinQ = sm.tile([P, JH, Q], BF16, name="BlinQ")
        BlinO = sm.tile([P, JH, Q], BF16, name="BlinO")
        for j in range(JH):
            nc.vector.tensor_scalar(
                BlinQ[:, j, :], rq_sb[:, j, :],
                WoCK_sb[:, j, M_HARM : M_HARM + 1], None, AluOpType.mult,
            )
            nc.vector.tensor_scalar(
                BlinO[:, j, :], onesQ[:], WoCK_sb[:, j, M_HARM : M_HARM + 1],
                None, AluOpType.mult,
            )

        # B-side harmonic products + folds (DVE; these are small)
        gB = sm.tile([P, M_HARM, 2, JH, Q], BF16, name="gB")

        def fold(m, t, src):
            for j in range(JH):
                nc.vector.tensor_scalar(
                    gB[:, m, t, j, :], src[:, j, :],
                    WoCK_sb[:, j, m : m + 1], None, AluOpType.mult,
                )

        fold(0, 0, sB)
        fold(0, 1, cB)
        s1sB = sm.tile([P, JH, Q], BF16, name="s1sB")
        nc.vector.tensor_tensor(s1sB[:], sB[:], sB[:], AluOpType.mult)
        s2B = sm.tile([P, JH, Q], BF16, name="s2B")
        nc.vector.tensor_tensor(s2B[:], sB[:], cB[:], AluOpType.mult)
        c2B = sm.tile([P, JH, Q], BF16, name="c2B")
        nc.vector.tensor_scalar(c2B[:], s1sB[:], -2.0, 1.0, AluOpType.mult, AluOpType.add)
        fold(1, 0, s2B)
        fold(1, 1, c2B)
        if M_HARM >= 3:
            c1sB = sm.tile([P, JH, Q], BF16, name="c1sB")
            nc.vector.tensor_tensor(c1sB[:], cB[:], cB[:], AluOpType.mult)
            u3B = sm.tile([P, JH, Q], BF16, name="u3B")
            nc.vector.tensor_scalar(u3B[:], s1sB[:], -4.0, 3.0, AluOpType.mult, AluOpType.add)
            s3B = sm.tile([P, JH, Q], BF16, name="s3B")
            nc.vector.tensor_tensor(s3B[:], u3B[:], sB[:], AluOpType.mult)
            v3B = sm.tile([P, JH, Q], BF16, name="v3B")
            nc.vector.tensor_scalar(v3B[:], c1sB[:], 4.0, -3.0, AluOpType.mult, AluOpType.add)
            c3B = sm.tile([P, JH, Q], BF16, name="c3B")
            nc.vector.tensor_tensor(c3B[:], v3B[:], cB[:], AluOpType.mult)
            fold(2, 0, s3B)
            fold(2, 1, c3B)
        if M_HARM >= 4:
            s2sB = sm.tile([P, JH, Q], BF16, name="s2sB")
            nc.vector.tensor_tensor(s2sB[:], s2B[:], s2B[:], AluOpType.mult)
            s4B = sm.tile([P, JH, Q], BF16, name="s4B")
            nc.vector.tensor_tensor(s4B[:], s2B[:], c2B[:], AluOpType.mult)
            c4B = sm.tile([P, JH, Q], BF16, name="c4B")
            nc.vector.tensor_scalar(c4B[:], s2sB[:], -8.0, 1.0, AluOpType.mult, AluOpType.add)
            fold(3, 0, s4B)
            fold(3, 1, c4B)

        # ---- A side (context, 512 cols): bias-free.  The sins read the
        # res_c PSUM directly (no staging dependency); the product chain is
        # split per h-chunk j so the PE's m0 chunks start one sin earlier.
        # rc_sb (needed only by the late BlinO contraction chunks) is
        # staged on ACT after the sins.
        wA = sm.tile([P, JH, C], F32, name="wA")
        sA = sm.tile([P, JH, C], BF16, name="sA")
        cA = sm.tile([P, JH, C], BF16, name="cA")
        s1sA = sm.tile([P, JH, C], BF16, name="s1sA")
        s2A = sm.tile([P, JH, C], BF16, name="s2A")
        c2A = sm.tile([P, JH, C], BF16, name="c2A")
        if M_HARM >= 3:
            c1sA = sm.tile([P, JH, C], BF16, name="c1sA")
            u3A = sm.tile([P, JH, C], BF16, name="u3A")
            s3A = sm.tile([P, JH, C], BF16, name="s3A")
            v3A = sm.tile([P, JH, C], BF16, name="v3A")
            c3A = sm.tile([P, JH, C], BF16, name="c3A")
        if M_HARM >= 4:
            s2sA = sm.tile([P, JH, C], BF16, name="s2sA")
            s4A = sm.tile([P, JH, C], BF16, name="s4A")
            c4A = sm.tile([P, JH, C], BF16, name="c4A")
        for j in range(JH):
            nc.vector.add_range_wrap(
                wA[:, j, :], rcp[:, j, :], (PI / 2) / W0, P0 / 2, P0
            )
            nc.scalar.activation(
                sA[:, j, :], rcp[:, j, :], Act.Sin, bias=0.0, scale=W0 * TRIM
            )
            nc.scalar.activation(
                cA[:, j, :], wA[:, j, :], Act.Sin, bias=0.0, scale=W0 * TRIM
            )
        rc_sb = sm.tile([P, JH, C], BF16, name="rc_sb")
        for j in range(JH):
            nc.scalar.activation(rc_sb[:, j, :], rcp[:, j, :], Act.Copy)
        for j in range(JH):
            nc.vector.tensor_tensor(
                s1sA[:, j, :], sA[:, j, :], sA[:, j, :], AluOpType.mult
            )
            nc.vector.tensor_tensor(
                s2A[:, j, :], sA[:, j, :], cA[:, j, :], AluOpType.mult
            )
            nc.vector.tensor_scalar(
                c2A[:, j, :], s1sA[:, j, :], -2.0, 1.0, AluOpType.mult, AluOpType.add
            )
            if M_HARM >= 3:
                nc.vector.tensor_tensor(
                    c1sA[:, j, :], cA[:, j, :], cA[:, j, :], AluOpType.mult
                )
                nc.vector.tensor_scalar(
                    u3A[:, j, :], s1sA[:, j, :], -4.0, 3.0, AluOpType.mult, AluOpType.add
                )
                nc.vector.tensor_tensor(
                    s3A[:, j, :], u3A[:, j, :], sA[:, j, :], AluOpType.mult
                )
                nc.vector.tensor_scalar(
                    v3A[:, j, :], c1sA[:, j, :], 4.0, -3.0, AluOpType.mult, AluOpType.add
                )
                nc.vector.tensor_tensor(
                    c3A[:, j, :], v3A[:, j, :], cA[:, j, :], AluOpType.mult
                )
            if M_HARM >= 4:
                nc.vector.tensor_tensor(
                    s2sA[:, j, :], s2A[:, j, :], s2A[:, j, :], AluOpType.mult
                )
                nc.vector.tensor_tensor(
                    s4A[:, j, :], s2A[:, j, :], c2A[:, j, :], AluOpType.mult
                )
                nc.vector.tensor_scalar(
                    c4A[:, j, :], s2sA[:, j, :], -8.0, 1.0, AluOpType.mult, AluOpType.add
                )

        # ---- logit contraction [q, c]: one PSUM bank, chunks ordered by
        # feature readiness (linear -> mask -> fundamental -> harmonics)
        lg = ps_lg.tile([Q, C], F32)
        first = dict(v=True)

        def mm(bt, at, stop=False):
            nc.tensor.matmul(lg[:], bt, at, start=first["v"], stop=stop)
            first["v"] = False

        for j in range(JH):
            mm(BlinQ[:, j, :], onesC[:])            # k*Wo.rq' broadcast over c
        mm(onesQ[0:1, :], mrow_sb[:])               # ln(mask) rank-1
        for j in range(JH):
            mm(gB[:, 0, 1, j, :], sA[:, j, :])
            mm(gB[:, 0, 0, j, :], cA[:, j, :])
        FA = [(s2A, c2A)]
        if M_HARM >= 3:
            FA.append((s3A, c3A))
        if M_HARM >= 4:
            FA.append((s4A, c4A))
        for j in range(JH):                         # m=2 (features ready early)
            mm(gB[:, 1, 1, j, :], FA[0][0][:, j, :])
            mm(gB[:, 1, 0, j, :], FA[0][1][:, j, :])
        for j in range(JH):
            mm(BlinO[:, j, :], rc_sb[:, j, :])      # k*Wo.rc broadcast over q
        for m in range(2, M_HARM):
            fAs, fAc = FA[m - 1]
            last = m == M_HARM - 1
            for j in range(JH):
                mm(gB[:, m, 1, j, :], fAs[:, j, :])
                mm(gB[:, m, 0, j, :], fAc[:, j, :], stop=(last and j == JH - 1))

        # ---- softmax tail: exp (+ masked row sums via accum_out), PE
        # transposes of the raw exp start immediately (no rowsum wait); the
        # 1/rowsum lands as a per-partition ACT scale on the final copies.
        expQ = sm.tile([Q, C], F32)
        sumQ = sm.tile([Q, 1], F32)
        nc.scalar.activation(
            expQ[:], lg[:], Act.Exp, bias=bo_sb[0:Q, 0:1], accum_out=sumQ[:]
        )
        for i in range(5):  # keep the PE p-state up through the exp wait
            nc.tensor.matmul(
                tp[0:Q, 0, :], warm[:], warm[:], start=True, stop=True
            )
        for k in range(KC):
            nc.tensor.transpose(
                tp[:, k, :], expQ[:, k * P : (k + 1) * P], ident[:]
            )
        eT_sb = sm.tile([P, KC, Q], BF16)
        nc.scalar.activation(eT_sb[:], tp[:], Act.Copy)
        # recQ chain emitted after eT so its ACT-side accumulator read does
        # not delay the eT staging on the in-order ACT queue
        recQ = sm.tile([Q, 1], F32)
        nc.vector.tensor_scalar_add(recQ[:], sumQ[:], float(EPS))
        nc.vector.reciprocal(recQ[:], recQ[:])
        ou = ps_ou.tile([Q, D], F32)
        for i in range(3):  # bridge the eT staging wait on a warm PE
            nc.tensor.matmul(
                tp[0:Q, 1, :], warm[:], warm[:], start=True, stop=True
            )
        for k in range(KC):
            nc.tensor.matmul(
                ou[:], eT_sb[:, k, :], ctx_sb[:, k, :],
                start=(k == 0), stop=(k == KC - 1),
            )
        w_sb = sm.tile([Q, C], F32)
        nc.vector.tensor_scalar(
            w_sb[:], expQ[:], recQ[:, 0:1], None, AluOpType.mult
        )
        nc.sync.dma_start(wts_d.ap()[:, :], w_sb[:])
        out_sb = sm.tile([Q, D], F32)
        nc.scalar.activation(out_sb[:], ou[:], Act.Copy, scale=recQ[:, 0:1])
        nc.sync.dma_start(out_d.ap()[:, :], out_sb[:])

    nc.compile()
    return nc


def _chunked(a, p=P):
    """[N*p, cols] -> [p, N*cols]: row k*p+i lands at [i, k*cols:(k+1)*cols]."""
    n = a.shape[0] // p
    return np.ascontiguousarray(
        a.reshape(n, p, a.shape[1]).transpose(1, 0, 2).reshape(p, n * a.shape[1])
    )


def make_in_maps(query, context, mask, W_c, b_c, W_q, W_o):
    import ml_dtypes
    f32 = np.float32
    bf16 = ml_dtypes.bfloat16
    WqT = _chunked(np.asarray(W_q, f32).T.astype(bf16))
    WcT = _chunked(np.asarray(W_c, f32).T.astype(bf16))
    Wo2 = np.asarray(W_o, f32).reshape(JH, P).T  # (P, JH)
    # the m=2 product feature is sin2/2, so its fold carries 2x
    cols = [f32(c) for c in CS]
    cols[1] = f32(2.0) * cols[1]
    if M_HARM >= 4:
        cols[3] = f32(4.0) * cols[3]
    cols.append(f32(K_LIN))
    WoCK = np.stack([Wo2 * c for c in cols], axis=2)  # (P, JH, M+1)
    bc2 = np.asarray(b_c, f32).reshape(JH, P).T[:, :, None]  # (P, JH, 1)
    WoCKB = np.ascontiguousarray(
        np.concatenate([WoCK, bc2], axis=2).astype(f32)
    )  # (P, JH, M+2)
    in_maps = []
    for b in range(B):
        mrow = np.asarray(mask[b], f32)
        mbr = np.maximum(np.log(np.maximum(mrow, 1e-300)), -50.0)
        in_maps.append(
            {
                "WqT": WqT,
                "WcT": WcT,
                "qT": _chunked(np.asarray(query[b], f32).T.astype(bf16)),
                "ctxT": _chunked(np.asarray(context[b], f32).T.astype(bf16)),
                "ctx": _chunked(np.asarray(context[b], bf16)),
                "mrow": np.ascontiguousarray(mbr.reshape(1, C).astype(bf16)),
                "WoCK": WoCKB,
            }
        )
    return in_maps


def kernel(query, context, mask, W_c, b_c, W_q, W_o, b_o):
    from concourse.bass_utils import run_bass_kernel_spmd

    nc = _build_program(float(np.asarray(b_o)))
    in_maps = make_in_maps(query, context, mask, W_c, b_c, W_q, W_o)
    res = run_bass_kernel_spmd(nc, in_maps, list(range(N_CORES))).results
    out = np.stack([res[b]["out"] for b in range(B)])
    wts = np.stack([res[b]["wts"] for b in range(B)])
    return out, wts


# revision 11
# speedup vs baseline: 1.7813x; 1.1371x over previous
"""Bass/Tile TRN2 kernel for nn_Attention (additive/Bahdanau-style attention).

reference math per batch b:
  res_q = query[b] @ W_q.T                      (Q, H)
  res_c = context[b] @ W_c.T + b_c              (C, H)
  logit[q,c] = sum_h W_o[h]*tanh(res_c[c,h] + res_q[q,h]) + b_o
  w = mask * exp(logit); weights = w / (sum_c w + eps)
  out = weights @ context[b]

The (Q,C,H) tanh grid is never materialized. tanh is replaced by an
M-term harmonic sine series  tanh(x) ~= k*x + sum_m c_m sin(m*w0*x), and
the angle-addition identity factorizes each term so the whole logit is
ONE PE contraction:  logit[q,c] = sum_f B_f[.,q] * A_f[.,c]   where
 - the harmonic features contract over (h, m, sin/cos) chunks of 128:
   only the fundamental sin/cos touch the ACT Sin table (cos through one
   DVE add_range_wrap with the pi/2 phase folded into the wrap shift);
   harmonic 2 comes from double-angle products, with the 2x of
   s2'=sin2/2 folded into the host-side W_o*c_m coefficients,
 - b_c folds into the rq staging copy (per-partition DVE bias), so the
   A(context) side is bias-free,
 - the linear k*Wo.rc term contracts over d against ctxT directly via
   the host-precomputed u = k*(Wo @ W_c), so res_c is never staged to
   bf16 (the sins read the f32 PSUM in place),
 - the linear k*Wo.rq' term broadcasts over c via a ones moving tile,
 - a rank-1 ln(mask) chunk folds the mask into the logit so exp's
   accum_out directly yields the masked softmax denominator.
The PE transposes of the raw exp start right after exp (no rowsum
dependency); 1/rowsum lands as a per-partition ACT scale on the final
output copy.

All matmul operands are bf16; PSUM accumulation stays f32.  Every bulk
input is pre-rearranged on the host into a [128, n*cols] partition-major
layout so each DMA moves few large contiguous lines.  Both HW-DGE
queues share one ~200GB/s DMA engine, so bulk rides the sync queue with
the CONTEXT side first: the critical chain is ctxT -> res_c -> sins ->
products -> last contraction chunks -> exp -> out, while the small
query side streams later and slots into the gaps.

Sharding: data-parallel over batch B=8 across the 8 NeuronCores.
"""

import numpy as np

B, Q, C, D, H = 8, 64, 512, 512, 256
EPS = 1e-5
P = 128
KD = D // P   # 4 chunks of the contraction dim d
KC = C // P   # 4 chunks of the context dim c
JH = H // P   # 2 chunks of the hidden dim h
N_CORES = 8

# tanh(x) ~= K_LIN*x + sum_m CS[m]*sin(m*W0*x), fitted on [-4.7, 4.7]
M_HARM = 2
W0 = 1.05
K_LIN = 0.3266410020214013
CS = [0.4380670801317152, 0.07800815282640118]
PI = float(np.pi)
TRIM = 1.0 - 5e-7    # keeps |w0*x| strictly inside the Sin table domain


def _build_program(b_o_val: float):
    import concourse.bacc as bacc
    import concourse.mybir as mybir
    import concourse.tile as tile
    from concourse.alu_op_type import AluOpType
    from concourse import masks
    from contextlib import ExitStack

    F32 = mybir.dt.float32
    BF16 = mybir.dt.bfloat16
    Act = mybir.ActivationFunctionType

    nc = bacc.Bacc("TRN2", target_bir_lowering=False, debug=False)

    # all bulk inputs arrive pre-rearranged: [P, k*cols], partition-major
    WqT_d = nc.dram_tensor("WqT", [P, KD * H], BF16, kind="ExternalInput")
    WcT_d = nc.dram_tensor("WcT", [P, KD * H], BF16, kind="ExternalInput")
    qT_d = nc.dram_tensor("qT", [P, KD * Q], BF16, kind="ExternalInput")
    ctxT_d = nc.dram_tensor("ctxT", [P, KD * C], BF16, kind="ExternalInput")
    ctx_d = nc.dram_tensor("ctx", [P, KC * D], BF16, kind="ExternalInput")
    mrow_d = nc.dram_tensor("mrow", [1, C], BF16, kind="ExternalInput")
    # cols 0..M-1: W_o * c_m folds (2x on m=2 for the halved sin2 product);
    # col M: W_o * k_lin; col M+1: b_c
    WoCK_d = nc.dram_tensor("WoCK", [P, JH, M_HARM + 2], F32, kind="ExternalInput")
    # u2[p, k] = (k_lin * W_o @ W_c)[k*128+p]: the linear context term
    # contracts over d against ctxT directly
    u2_d = nc.dram_tensor("u2", [P, KD], F32, kind="ExternalInput")
    out_d = nc.dram_tensor("out", [Q, D], F32, kind="ExternalOutput")
    wts_d = nc.dram_tensor("wts", [Q, C], F32, kind="ExternalOutput")

    with tile.TileContext(nc) as tc, ExitStack() as ctx:
        const = ctx.enter_context(tc.tile_pool(name="const", bufs=1))
        sm = ctx.enter_context(tc.tile_pool(name="sm", bufs=1))
        ps_rc = ctx.enter_context(tc.tile_pool(name="ps_rc", bufs=1, space="PSUM"))
        ps_rq = ctx.enter_context(tc.tile_pool(name="ps_rq", bufs=1, space="PSUM"))
        ps_lg = ctx.enter_context(tc.tile_pool(name="ps_lg", bufs=1, space="PSUM"))
        ps_tp = ctx.enter_context(tc.tile_pool(name="ps_tp", bufs=1, space="PSUM"))
        ps_ou = ctx.enter_context(tc.tile_pool(name="ps_ou", bufs=1, space="PSUM"))

        # ---- input DMAs: context side first on the sync queue
        WqT_sb = const.tile([P, KD, H], BF16)
        WcT_sb = const.tile([P, KD, H], BF16)
        qT_sb = const.tile([P, KD, Q], BF16)
        ctxT_sb = const.tile([P, KD, C], BF16)
        WoCK_sb = const.tile([P, JH, M_HARM + 2], F32)
        u2_sb = const.tile([P, KD], F32)
        mrow_sb = const.tile([1, C], BF16)
        nc.sync.dma_start(WcT_sb[:], WcT_d.ap())
        nc.sync.dma_start(ctxT_sb[:, 0:2, :], ctxT_d.ap()[:, 0 : 2 * C])
        nc.sync.dma_start(ctxT_sb[:, 2:4, :], ctxT_d.ap()[:, 2 * C : 4 * C])
        nc.sync.dma_start(qT_sb[:], qT_d.ap())
        nc.sync.dma_start(WqT_sb[:], WqT_d.ap())
        nc.scalar.dma_start(WoCK_sb[:], WoCK_d.ap())
        nc.scalar.dma_start(u2_sb[:], u2_d.ap())
        nc.scalar.dma_start(mrow_sb[:], mrow_d.ap())
        # ctx (only needed by the final weights @ ctx matmul) is triggered
        # after the res matmuls are emitted: readers of any earlier tensor
        # conservatively wait on all previously-issued DMAs
        ctx_sb = const.tile([P, KC, D], BF16)

        onesC = const.tile([P, C], BF16)
        nc.gpsimd.memset(onesC[:], 1.0)
        onesQ = const.tile([P, Q], BF16)
        nc.gpsimd.memset(onesQ[:], 1.0)
        ident = const.tile([Q, Q], F32)
        masks.make_identity(nc, ident[:])
        bo_sb = const.tile([P, 1], F32)
        nc.vector.memset(bo_sb[:], float(b_o_val))
        # dummy first ACT op: forces the trig table (which also contains
        # Identity/Copy/Square) to be the one loaded during the DMA stream
        warm = const.tile([P, Q], BF16)
        nc.vector.memset(warm[:], 0.25)
        sinwarm = sm.tile([1, 1], BF16, name="sinwarm")
        nc.scalar.activation(sinwarm[:], warm[0:1, 0:1], Act.Sin, bias=0.0, scale=W0)
        # Bu[p, k, q] = u2[p, k] broadcast over q (stationary for the
        # linear-context chunks)
        Bu = sm.tile([P, KD, Q], BF16, name="Bu")
        for k in range(KD):
            nc.vector.tensor_scalar(
                Bu[:, k, :], onesQ[:], u2_sb[:, k : k + 1], None, AluOpType.mult
            )

        # ---- PE warmup junk while the context stream lands; then res_c
        # (k-outer, chasing the two ctxT DMA chunks) interleaved with the
        # linear-context lg chunks, then res_q as soon as its inputs land.
        rcp = ps_rc.tile([P, JH, C], F32)
        rqp = ps_rq.tile([P, JH, Q], F32)
        tp = ps_tp.tile([P, KC, Q], F32)
        lg = ps_lg.tile([Q, C], F32)
        for i in range(60):
            nc.tensor.matmul(
                tp[0:Q, 0, :], warm[:], warm[:], start=True, stop=True
            )
        first = dict(v=True)

        def mm(bt, at, stop=False):
            nc.tensor.matmul(lg[:], bt, at, start=first["v"], stop=stop)
            first["v"] = False

        for k in range(KD):
            for j in range(JH):
                hs = slice(j * P, (j + 1) * P)
                nc.tensor.matmul(
                    rcp[:, j, :], WcT_sb[:, k, hs], ctxT_sb[:, k, :],
                    start=(k == 0), stop=(k == KD - 1),
                )
            mm(Bu[:, k, :], ctxT_sb[:, k, :])       # k_lin*Wo.rc over d
        for j in range(JH):
            hs = slice(j * P, (j + 1) * P)
            for k in range(KD):
                nc.tensor.matmul(
                    rqp[:, j, :], WqT_sb[:, k, hs], qT_sb[:, k, :],
                    start=(k == 0), stop=(k == KD - 1),
                )
        nc.sync.dma_start(ctx_sb[:], ctx_d.ap())

        # ---- A side (context, 512 cols): the sins read the res_c PSUM
        # directly, split per h-chunk j so the PE's m0 chunks start one
        # sin earlier; products on the DVE.
        P0 = 2.0 * PI / W0
        wA = sm.tile([P, JH, C], F32, name="wA")
        sA = sm.tile([P, JH, C], BF16, name="sA")
        cA = sm.tile([P, JH, C], BF16, name="cA")
        s1sA = sm.tile([P, JH, C], BF16, name="s1sA")
        s2A = sm.tile([P, JH, C], BF16, name="s2A")
        c2A = sm.tile([P, JH, C], BF16, name="c2A")
        for j in range(JH):
            nc.vector.add_range_wrap(
                wA[:, j, :], rcp[:, j, :], (PI / 2) / W0, P0 / 2, P0
            )
        # B-side staging (b_c enters as the per-partition bias) runs on the
        # DVE; the B sins slot into the ACT queue between the A sins
        rq_sb = sm.tile([P, JH, Q], BF16, name="rq_sb")
        for j in range(JH):
            nc.vector.tensor_scalar(
                rq_sb[:, j, :], rqp[:, j, :],
                WoCK_sb[:, j, M_HARM + 1 : M_HARM + 2], None, AluOpType.add,
            )
        wB = sm.tile([P, JH, Q], F32, name="wB")
        nc.vector.add_range_wrap(wB[:], rq_sb[:], (PI / 2) / W0, P0 / 2, P0)
        sB = sm.tile([P, JH, Q], BF16, name="sB")
        cB = sm.tile([P, JH, Q], BF16, name="cB")

        nc.scalar.activation(
            sA[:, 0, :], rcp[:, 0, :], Act.Sin, bias=0.0, scale=W0 * TRIM
        )
        nc.scalar.activation(
            cA[:, 0, :], wA[:, 0, :], Act.Sin, bias=0.0, scale=W0 * TRIM
        )
        nc.scalar.activation(sB[:], rq_sb[:], Act.Sin, bias=0.0, scale=W0 * TRIM)
        nc.scalar.activation(cB[:], wB[:], Act.Sin, bias=0.0, scale=W0 * TRIM)
        nc.scalar.activation(
            sA[:, 1, :], rcp[:, 1, :], Act.Sin, bias=0.0, scale=W0 * TRIM
        )
        nc.scalar.activation(
            cA[:, 1, :], wA[:, 1, :], Act.Sin, bias=0.0, scale=W0 * TRIM
        )

        # DVE: A products j0, then B chain + folds, then A products j1
        def a_products(j):
            nc.vector.tensor_tensor(
                s1sA[:, j, :], sA[:, j, :], sA[:, j, :], AluOpType.mult
            )
            nc.vector.tensor_tensor(
                s2A[:, j, :], sA[:, j, :], cA[:, j, :], AluOpType.mult
            )
            nc.vector.tensor_scalar(
                c2A[:, j, :], s1sA[:, j, :], -2.0, 1.0, AluOpType.mult, AluOpType.add
            )

        a_products(0)
        BlinQ = sm.tile([P, JH, Q], BF16, name="BlinQ")
        for j in range(JH):
            nc.vector.tensor_scalar(
                BlinQ[:, j, :], rq_sb[:, j, :],
                WoCK_sb[:, j, M_HARM : M_HARM + 1], None, AluOpType.mult,
            )
        gB = sm.tile([P, M_HARM, 2, JH, Q], BF16, name="gB")

        def fold(m, t, src):
            for j in range(JH):
                nc.vector.tensor_scalar(
                    gB[:, m, t, j, :], src[:, j, :],
                    WoCK_sb[:, j, m : m + 1], None, AluOpType.mult,
                )

        fold(0, 0, sB)
        fold(0, 1, cB)
        s1sB = sm.tile([P, JH, Q], BF16, name="s1sB")
        nc.vector.tensor_tensor(s1sB[:], sB[:], sB[:], AluOpType.mult)
        s2B = sm.tile([P, JH, Q], BF16, name="s2B")
        nc.vector.tensor_tensor(s2B[:], sB[:], cB[:], AluOpType.mult)
        c2B = sm.tile([P, JH, Q], BF16, name="c2B")
        nc.vector.tensor_scalar(c2B[:], s1sB[:], -2.0, 1.0, AluOpType.mult, AluOpType.add)
        fold(1, 0, s2B)
        fold(1, 1, c2B)
        a_products(1)

        # ---- remaining logit chunks, ordered by feature readiness
        mm(onesQ[0:1, :], mrow_sb[:])               # ln(mask) rank-1
        for j in range(JH):
            mm(BlinQ[:, j, :], onesC[:])            # k*Wo.rq' broadcast over c
        for i in range(26):  # p-state bridge over the feature wait
            nc.tensor.matmul(
                tp[0:Q, 1, :], warm[:], warm[:], start=True, stop=True
            )
        mm(gB[:, 0, 1, 0, :], sA[:, 0, :])
        mm(gB[:, 0, 0, 0, :], cA[:, 0, :])
        mm(gB[:, 1, 1, 0, :], s2A[:, 0, :])
        mm(gB[:, 1, 0, 0, :], c2A[:, 0, :])
        mm(gB[:, 0, 1, 1, :], sA[:, 1, :])
        mm(gB[:, 0, 0, 1, :], cA[:, 1, :])
        mm(gB[:, 1, 1, 1, :], s2A[:, 1, :])
        mm(gB[:, 1, 0, 1, :], c2A[:, 1, :], stop=True)

        # ---- softmax tail: exp (+ masked row sums via accum_out), PE
        # transposes of the raw exp start immediately (no rowsum wait); the
        # 1/rowsum lands as a per-partition ACT scale on the final copy.
        expQ = sm.tile([Q, C], F32)
        sumQ = sm.tile([Q, 1], F32)
        nc.scalar.activation(
            expQ[:], lg[:], Act.Exp, bias=bo_sb[0:Q, 0:1], accum_out=sumQ[:]
        )
        for i in range(5):  # keep the PE p-state up through the exp wait
            nc.tensor.matmul(
                tp[0:Q, 0, :], warm[:], warm[:], start=True, stop=True
            )
        for k in range(KC):
            nc.tensor.transpose(
                tp[:, k, :], expQ[:, k * P : (k + 1) * P], ident[:]
            )
        eT_sb = sm.tile([P, KC, Q], BF16)
        nc.scalar.activation(eT_sb[:], tp[:], Act.Copy)
        # recQ chain emitted after eT so its ACT-side accumulator read does
        # not delay the eT staging on the in-order ACT queue
        recQ = sm.tile([Q, 1], F32)
        nc.vector.tensor_scalar_add(recQ[:], sumQ[:], float(EPS))
        nc.vector.reciprocal(recQ[:], recQ[:])
        w_sb = sm.tile([Q, C], F32)
        nc.vector.tensor_scalar(
            w_sb[:], expQ[:], recQ[:, 0:1], None, AluOpType.mult
        )
        nc.sync.dma_start(wts_d.ap()[:, :], w_sb[:])
        ou = ps_ou.tile([Q, D], F32)
        for i in range(3):  # bridge the eT staging wait on a warm PE
            nc.tensor.matmul(
                tp[0:Q, 1, :], warm[:], warm[:], start=True, stop=True
            )
        for k in range(KC):
            nc.tensor.matmul(
                ou[:], eT_sb[:, k, :], ctx_sb[:, k, :],
                start=(k == 0), stop=(k == KC - 1),
            )
        out_sb = sm.tile([Q, D], F32)
        nc.scalar.activation(out_sb[:], ou[:], Act.Copy, scale=recQ[:, 0:1])
        nc.sync.dma_start(out_d.ap()[:, :], out_sb[:])

    nc.compile()
    return nc


def _chunked(a, p=P):
    """[N*p, cols] -> [p, N*cols]: row k*p+i lands at [i, k*cols:(k+1)*cols]."""
    n = a.shape[0] // p
    return np.ascontiguousarray(
        a.reshape(n, p, a.shape[1]).transpose(1, 0, 2).reshape(p, n * a.shape[1])
    )


def make_in_maps(query, context, mask, W_c, b_c, W_q, W_o):
    import ml_dtypes
    f32 = np.float32
    bf16 = ml_dtypes.bfloat16
    WqT = _chunked(np.asarray(W_q, f32).T.astype(bf16))
    WcT = _chunked(np.asarray(W_c, f32).T.astype(bf16))
    Wo2 = np.asarray(W_o, f32).reshape(JH, P).T  # (P, JH)
    # the m=2 product feature is sin2/2, so its fold carries 2x
    cols = [f32(c) for c in CS]
    cols[1] = f32(2.0) * cols[1]
    cols.append(f32(K_LIN))
    WoCK = np.stack([Wo2 * c for c in cols], axis=2)  # (P, JH, M+1)
    bc2 = np.asarray(b_c, f32).reshape(JH, P).T[:, :, None]  # (P, JH, 1)
    WoCKB = np.ascontiguousarray(
        np.concatenate([WoCK, bc2], axis=2).astype(f32)
    )  # (P, JH, M+2)
    u = f32(K_LIN) * (np.asarray(W_o, f32) @ np.asarray(W_c, f32))  # (D,)
    u2 = np.ascontiguousarray(u.reshape(KD, P).T.astype(f32))  # (P, KD)
    in_maps = []
    for b in range(B):
        mrow = np.asarray(mask[b], f32)
        mbr = np.maximum(np.log(np.maximum(mrow, 1e-300)), -50.0)
        in_maps.append(
            {
                "WqT": WqT,
                "WcT": WcT,
                "qT": _chunked(np.asarray(query[b], f32).T.astype(bf16)),
                "ctxT": _chunked(np.asarray(context[b], f32).T.astype(bf16)),
                "ctx": _chunked(np.asarray(context[b], bf16)),
                "mrow": np.ascontiguousarray(mbr.reshape(1, C).astype(bf16)),
                "WoCK": WoCKB,
                "u2": u2,
            }
        )
    return in_maps


def kernel(query, context, mask, W_c, b_c, W_q, W_o, b_o):
    from concourse.bass_utils import run_bass_kernel_spmd

    nc = _build_program(float(np.asarray(b_o)))
    in_maps = make_in_maps(query, context, mask, W_c, b_c, W_q, W_o)
    res = run_bass_kernel_spmd(nc, in_maps, list(range(N_CORES))).results
    out = np.stack([res[b]["out"] for b in range(B)])
    wts = np.stack([res[b]["wts"] for b in range(B)])
    return out, wts
